# revision 3
# baseline (speedup 1.0000x reference)
"""N-gram embedding lookup (mean of hashed n-gram embeddings per word) on 8
Trainium2 NeuronCores.

Data-parallel sharding: the 16x2048 word_idx grid is flattened and split into
8 chunks of 4096 words; the n-gram id table and embedding table are replicated.

Per core, per block of 128 words (32 blocks):
  1. one indirect DMA gathers the words' meta rows (24 n-gram ids + f32 bits
     of 1/count, packed as one int32[32] row per word) using the word indices
     as offsets,
  2. one indirect DMA gathers all 128*24 embedding rows (512B each) straight
     from HBM into SBUF, laid out word-on-partition / (k,e) on the free dim,
  3. VectorE reduces over k and multiplies by 1/count,
  4. the [128,128] result is stored to the output.

Padding slots carry n-gram id 0 and the embedding table's row 0 is zeroed
host-side (nn.Embedding padding_idx=0), so the plain sum equals the masked sum.
"""

import numpy as np

import concourse.bass as bass
import concourse.tile as tile
from concourse import bacc, mybir
from concourse.bass_utils import run_bass_kernel_spmd

N_CORES = 8
B, S, E = 16, 2048, 128
V, K, NG = 32000, 24, 200000
MW = 32  # meta row width (ints per word): 24 ids + 1 recip + pad
N_PER_CORE = B * S // N_CORES  # 4096
NBLK = N_PER_CORE // 128  # 32

_cache = {}


def _build():
    if "nc" in _cache:
        return _cache["nc"]
    nc = bacc.Bacc("TRN2", target_bir_lowering=False, debug=False, num_devices=N_CORES)
    widx = nc.dram_tensor("widx", [128, NBLK], mybir.dt.int32, kind="ExternalInput").ap()
    meta = nc.dram_tensor("meta", [V, MW], mybir.dt.int32, kind="ExternalInput").ap()
    table = nc.dram_tensor("table", [NG, E], mybir.dt.float32, kind="ExternalInput").ap()
    out = nc.dram_tensor(
        "out", [N_PER_CORE, E], mybir.dt.float32, kind="ExternalOutput"
    ).ap()

    with tile.TileContext(nc) as tc:
        with (
            tc.tile_pool(name="widxp", bufs=1) as widxp,
            tc.tile_pool(name="metap", bufs=1) as metap,
            tc.tile_pool(name="gp", bufs=4) as gp,
            tc.tile_pool(name="redp", bufs=4) as redp,
            tc.tile_pool(name="outp", bufs=4) as outp,
        ):
            widx_t = widxp.tile([128, NBLK], mybir.dt.int32)
            nc.sync.dma_start(widx_t[:], widx[:])

            meta_t = metap.tile([128, NBLK * MW], mybir.dt.int32)
            for j in range(NBLK):
                nc.gpsimd.indirect_dma_start(
                    out=meta_t[:, j * MW : (j + 1) * MW],
                    out_offset=None,
                    in_=meta[:],
                    in_offset=bass.IndirectOffsetOnAxis(
                        ap=widx_t[:, j : j + 1], axis=0
                    ),
                )

            for j in range(NBLK):
                g = gp.tile([128, K * E], mybir.dt.float32, tag="g")
                for k in range(K):
                    nc.gpsimd.indirect_dma_start(
                        out=g[:, k * E : (k + 1) * E],
                        out_offset=None,
                        in_=table[:],
                        in_offset=bass.IndirectOffsetOnAxis(
                            ap=meta_t[:, j * MW + k : j * MW + k + 1], axis=0
                        ),
                    )
                red = redp.tile([128, E], mybir.dt.float32, tag="red")
                nc.vector.tensor_reduce(
                    out=red[:],
                    in_=g[:].rearrange("p (k e) -> p e k", k=K, e=E),
                    axis=mybir.AxisListType.X,
                    op=mybir.AluOpType.add,
                )
                o = outp.tile([128, E], mybir.dt.float32, tag="o")
                nc.vector.tensor_scalar_mul(
                    out=o[:],
                    in0=red[:],
                    scalar1=meta_t[:, j * MW + K : j * MW + K + 1].bitcast(
                        mybir.dt.float32
                    ),
                )
                nc.sync.dma_start(out[j * 128 : (j + 1) * 128, :], o[:])

    nc.compile()
    _cache["nc"] = nc
    return nc


def _prep_shared(ngram_ids, ngram_counts, emb_table):
    meta_np = np.zeros((V, MW), dtype=np.int32)
    meta_np[:, :K] = ngram_ids
    meta_np[:, K] = (1.0 / ngram_counts.astype(np.float32)).view(np.int32)
    table_np = np.ascontiguousarray(emb_table, dtype=np.float32).copy()
    table_np[0, :] = 0.0
    return meta_np, table_np


def kernel(word_idx, ngram_ids, ngram_counts, emb_table):
    word_idx = np.asarray(word_idx, dtype=np.int32)
    ngram_ids = np.asarray(ngram_ids, dtype=np.int32)
    ngram_counts = np.asarray(ngram_counts, dtype=np.int32)
    emb_table = np.asarray(emb_table, dtype=np.float32)

    nc = _build()
    meta_np, table_np = _prep_shared(ngram_ids, ngram_counts, emb_table)

    flat = word_idx.reshape(-1)
    in_maps = []
    for c in range(N_CORES):
        chunk = flat[c * N_PER_CORE : (c + 1) * N_PER_CORE]
        widx_np = np.ascontiguousarray(chunk.reshape(NBLK, 128).T)
        in_maps.append({"widx": widx_np, "meta": meta_np, "table": table_np})

    res = run_bass_kernel_spmd(nc, in_maps, list(range(N_CORES)))
    outs = [res.results[c]["out"] for c in range(N_CORES)]
    return np.concatenate(outs, axis=0).reshape(B, S, E)


# revision 5
# speedup vs baseline: 1.5244x; 1.5244x over previous
"""N-gram embedding lookup (mean of hashed n-gram embeddings per word) on 8
Trainium2 NeuronCores.

Data-parallel sharding: the 16x2048 word_idx grid is flattened and split
across 8 cores; the n-gram id table and embedding table are replicated.

The SWDGE indirect-DMA primitive gathers 128 rows (one index per partition)
per instruction at a fixed ~1.4us instruction cost, so instruction count is
the bottleneck. To minimize it, words are sorted by their n-gram count and
dealt to cores in sorted stripes of 8 blocks x 128 words: every block then
only gathers K_i = max-count-in-stripe embedding rows per word instead of
the full 24, and the per-block K_i schedule is compiled into the program
(identical across cores; SPMD). Unused slots carry n-gram id 0 whose
embedding row is zeroed host-side (padding_idx=0), so short sums are exact.

Per core, per block of 128 words (32 blocks):
  1. one indirect DMA gathers the words' meta rows (24 n-gram ids + f32 bits
     of 1/count packed as one int32[32] row per word),
  2. K_i indirect DMAs gather the 128*K_i embedding rows (512B each) from
     HBM into SBUF, word-on-partition / (k,e) on the free dim,
  3. VectorE reduces over k and multiplies by 1/count,
  4. the [128,128] result is stored; the host undoes the sort permutation.
"""

import numpy as np

import concourse.bass as bass
import concourse.tile as tile
from concourse import bacc, mybir
from concourse.bass_utils import run_bass_kernel_spmd

N_CORES = 8
B, S, E = 16, 2048, 128
V, K, NG = 32000, 24, 200000
MW = 32  # meta row width (ints per word): 24 ids + 1 recip + pad
NW = B * S  # 32768 words
N_PER_CORE = NW // N_CORES  # 4096
NBLK = N_PER_CORE // 128  # 32 blocks per core
STRIPE = N_CORES * 128  # words per stripe (one block per core)

_cache = {}


def _build(k_list):
    key = tuple(k_list)
    if key in _cache:
        return _cache[key]
    nc = bacc.Bacc("TRN2", target_bir_lowering=False, debug=False, num_devices=N_CORES)
    widx = nc.dram_tensor("widx", [128, NBLK], mybir.dt.int32, kind="ExternalInput").ap()
    meta = nc.dram_tensor("meta", [V, MW], mybir.dt.int32, kind="ExternalInput").ap()
    table = nc.dram_tensor("table", [NG, E], mybir.dt.float32, kind="ExternalInput").ap()
    out = nc.dram_tensor(
        "out", [N_PER_CORE, E], mybir.dt.float32, kind="ExternalOutput"
    ).ap()

    with tile.TileContext(nc) as tc:
        with (
            tc.tile_pool(name="widxp", bufs=1) as widxp,
            tc.tile_pool(name="metap", bufs=1) as metap,
            tc.tile_pool(name="gp", bufs=4) as gp,
            tc.tile_pool(name="redp", bufs=4) as redp,
            tc.tile_pool(name="outp", bufs=4) as outp,
        ):
            widx_t = widxp.tile([128, NBLK], mybir.dt.int32)
            nc.sync.dma_start(widx_t[:], widx[:])

            meta_t = metap.tile([128, NBLK * MW], mybir.dt.int32)
            for j in range(NBLK):
                nc.gpsimd.indirect_dma_start(
                    out=meta_t[:, j * MW : (j + 1) * MW],
                    out_offset=None,
                    in_=meta[:],
                    in_offset=bass.IndirectOffsetOnAxis(
                        ap=widx_t[:, j : j + 1], axis=0
                    ),
                )

            for j in range(NBLK):
                kj = k_list[j]
                g = gp.tile([128, K * E], mybir.dt.float32, tag="g")
                for k in range(kj):
                    nc.gpsimd.indirect_dma_start(
                        out=g[:, k * E : (k + 1) * E],
                        out_offset=None,
                        in_=table[:],
                        in_offset=bass.IndirectOffsetOnAxis(
                            ap=meta_t[:, j * MW + k : j * MW + k + 1], axis=0
                        ),
                    )
                red = redp.tile([128, E], mybir.dt.float32, tag="red")
                if kj > 1:
                    nc.vector.tensor_reduce(
                        out=red[:],
                        in_=g[:, : kj * E].rearrange("p (k e) -> p e k", k=kj, e=E),
                        axis=mybir.AxisListType.X,
                        op=mybir.AluOpType.add,
                    )
                    src = red
                else:
                    src = g
                o = outp.tile([128, E], mybir.dt.float32, tag="o")
                nc.vector.tensor_scalar_mul(
                    out=o[:],
                    in0=src[:, :E],
                    scalar1=meta_t[:, j * MW + K : j * MW + K + 1].bitcast(
                        mybir.dt.float32
                    ),
                )
                nc.sync.dma_start(out[j * 128 : (j + 1) * 128, :], o[:])

    nc.compile()
    _cache[key] = nc
    return nc


def _prep_shared(ngram_ids, ngram_counts, emb_table):
    meta_np = np.zeros((V, MW), dtype=np.int32)
    meta_np[:, :K] = ngram_ids
    meta_np[:, K] = (1.0 / ngram_counts.astype(np.float32)).view(np.int32)
    table_np = np.ascontiguousarray(emb_table, dtype=np.float32).copy()
    table_np[0, :] = 0.0
    return meta_np, table_np


def _prepare(word_idx, ngram_ids, ngram_counts, emb_table):
    """Compile (count-schedule-specific) program + build per-core inputs."""
    word_idx = np.asarray(word_idx, dtype=np.int32)
    ngram_ids = np.asarray(ngram_ids, dtype=np.int32)
    ngram_counts = np.asarray(ngram_counts, dtype=np.int32)
    emb_table = np.asarray(emb_table, dtype=np.float32)

    meta_np, table_np = _prep_shared(ngram_ids, ngram_counts, emb_table)

    flat = word_idx.reshape(-1)
    cnts = ngram_counts[flat]
    order = np.argsort(-cnts, kind="stable")
    sorted_words = flat[order]
    # stripe i = global blocks [i*8, i*8+8); core c gets block i*8+c as its
    # i-th block. The compiled K for block-slot i is the stripe max = the
    # count of the stripe's first word (descending sort).
    k_list = [int(cnts[order[i * STRIPE]]) for i in range(NBLK)]

    nc = _build(k_list)

    in_maps = []
    for c in range(N_CORES):
        w = np.empty((128, NBLK), dtype=np.int32)
        for i in range(NBLK):
            g0 = (i * N_CORES + c) * 128
            w[:, i] = sorted_words[g0 : g0 + 128]
        in_maps.append({"widx": w, "meta": meta_np, "table": table_np})
    return nc, in_maps, order


def _assemble(results, order):
    out_sorted = np.empty((NW, E), dtype=np.float32)
    for c in range(N_CORES):
        oc = results[c]["out"]
        for i in range(NBLK):
            g0 = (i * N_CORES + c) * 128
            out_sorted[g0 : g0 + 128] = oc[i * 128 : (i + 1) * 128]
    result = np.empty((NW, E), dtype=np.float32)
    result[order] = out_sorted
    return result.reshape(B, S, E)


def kernel(word_idx, ngram_ids, ngram_counts, emb_table):
    nc, in_maps, order = _prepare(word_idx, ngram_ids, ngram_counts, emb_table)
    res = run_bass_kernel_spmd(nc, in_maps, list(range(N_CORES)))
    return _assemble(res.results, order)


# revision 7
# speedup vs baseline: 1.7724x; 1.1627x over previous
"""N-gram embedding lookup (mean of hashed n-gram embeddings per word) on 8
Trainium2 NeuronCores.

Data-parallel sharding: the 16x2048 word_idx grid is flattened and split
across 8 cores; the n-gram id table and embedding table are replicated.

The SWDGE indirect-DMA primitive gathers 128 rows (one index per partition)
per instruction at a fixed ~1.4us instruction cost, so instruction count is
the bottleneck. To minimize it, words are sorted by their n-gram count and
dealt to cores in sorted stripes of 8 blocks x 128 words: every block then
only gathers K_i = max-count-in-stripe embedding rows per word instead of
the full 24, and the per-block K_i schedule is compiled into the program
(identical across cores; SPMD). Unused slots carry n-gram id 0 whose
embedding row is zeroed host-side (padding_idx=0), so short sums are exact.

Per core, per block of 128 words (32 blocks):
  1. one indirect DMA gathers the words' meta rows (24 n-gram ids + f32 bits
     of 1/count packed as one int32[32] row per word),
  2. K_i indirect DMAs gather the 128*K_i embedding rows (512B each) from
     HBM into SBUF, word-on-partition / (k,e) on the free dim,
  3. VectorE reduces over k and multiplies by 1/count,
  4. the [128,128] result is stored; the host undoes the sort permutation.
"""

import numpy as np

import concourse.bass as bass
import concourse.tile as tile
from concourse import bacc, mybir
from concourse.bass_utils import run_bass_kernel_spmd

N_CORES = 8
B, S, E = 16, 2048, 128
V, K, NG = 32000, 24, 200000
MW = 32  # meta row width (ints per word): 24 ids + 1 recip + pad
NW = B * S  # 32768 words
N_PER_CORE = NW // N_CORES  # 4096
NBLK = N_PER_CORE // 128  # 32 blocks per core
STRIPE = N_CORES * 128  # words per stripe (one block per core)

_cache = {}


def _build(k_list):
    key = tuple(k_list)
    if key in _cache:
        return _cache[key]
    nc = bacc.Bacc("TRN2", target_bir_lowering=False, debug=False, num_devices=N_CORES)
    widx = nc.dram_tensor("widx", [128, NBLK], mybir.dt.int32, kind="ExternalInput").ap()
    meta = nc.dram_tensor("meta", [V, MW], mybir.dt.int32, kind="ExternalInput").ap()
    table = nc.dram_tensor("table", [NG, E], mybir.dt.float32, kind="ExternalInput").ap()
    out = nc.dram_tensor(
        "out", [N_PER_CORE, E], mybir.dt.float32, kind="ExternalOutput"
    ).ap()

    with tile.TileContext(nc) as tc:
        with (
            tc.tile_pool(name="widxp", bufs=1) as widxp,
            tc.tile_pool(name="metap", bufs=1) as metap,
            tc.tile_pool(name="gp", bufs=6) as gp,
            tc.tile_pool(name="outp", bufs=4) as outp,
        ):
            widx_t = widxp.tile([128, NBLK], mybir.dt.int32)
            nc.sync.dma_start(widx_t[:], widx[:])

            meta_t = metap.tile([128, NBLK * MW], mybir.dt.int32)
            for j in range(NBLK):
                nc.gpsimd.indirect_dma_start(
                    out=meta_t[:, j * MW : (j + 1) * MW],
                    out_offset=None,
                    in_=meta[:],
                    in_offset=bass.IndirectOffsetOnAxis(
                        ap=widx_t[:, j : j + 1], axis=0
                    ),
                )

            for j in range(NBLK):
                kj = k_list[j]
                g = gp.tile([128, K * E], mybir.dt.float32, tag="g")
                for k in range(kj):
                    nc.gpsimd.indirect_dma_start(
                        out=g[:, k * E : (k + 1) * E],
                        out_offset=None,
                        in_=table[:],
                        in_offset=bass.IndirectOffsetOnAxis(
                            ap=meta_t[:, j * MW + k : j * MW + k + 1], axis=0
                        ),
                    )
                # contiguous halving-tree sum over the kj slots, in place in g
                m = kj
                while m > 1:
                    h = m // 2
                    nc.vector.tensor_tensor(
                        out=g[:, 0 : h * E],
                        in0=g[:, 0 : h * E],
                        in1=g[:, (m - h) * E : m * E],
                        op=mybir.AluOpType.add,
                    )
                    m -= h
                o = outp.tile([128, E], mybir.dt.float32, tag="o")
                nc.vector.tensor_scalar_mul(
                    out=o[:],
                    in0=g[:, :E],
                    scalar1=meta_t[:, j * MW + K : j * MW + K + 1].bitcast(
                        mybir.dt.float32
                    ),
                )
                nc.sync.dma_start(out[j * 128 : (j + 1) * 128, :], o[:])

    nc.compile()
    _cache[key] = nc
    return nc


def _prep_shared(ngram_ids, ngram_counts, emb_table):
    meta_np = np.zeros((V, MW), dtype=np.int32)
    meta_np[:, :K] = ngram_ids
    meta_np[:, K] = (1.0 / ngram_counts.astype(np.float32)).view(np.int32)
    table_np = np.ascontiguousarray(emb_table, dtype=np.float32).copy()
    table_np[0, :] = 0.0
    return meta_np, table_np


def _prepare(word_idx, ngram_ids, ngram_counts, emb_table):
    """Compile (count-schedule-specific) program + build per-core inputs."""
    word_idx = np.asarray(word_idx, dtype=np.int32)
    ngram_ids = np.asarray(ngram_ids, dtype=np.int32)
    ngram_counts = np.asarray(ngram_counts, dtype=np.int32)
    emb_table = np.asarray(emb_table, dtype=np.float32)

    meta_np, table_np = _prep_shared(ngram_ids, ngram_counts, emb_table)

    flat = word_idx.reshape(-1)
    cnts = ngram_counts[flat]
    order = np.argsort(-cnts, kind="stable")
    sorted_words = flat[order]
    # stripe i = global blocks [i*8, i*8+8); core c gets block i*8+c as its
    # i-th block. The compiled K for block-slot i is the stripe max = the
    # count of the stripe's first word (descending sort).
    k_list = [int(cnts[order[i * STRIPE]]) for i in range(NBLK)]

    nc = _build(k_list)

    in_maps = []
    for c in range(N_CORES):
        w = np.empty((128, NBLK), dtype=np.int32)
        for i in range(NBLK):
            g0 = (i * N_CORES + c) * 128
            w[:, i] = sorted_words[g0 : g0 + 128]
        in_maps.append({"widx": w, "meta": meta_np, "table": table_np})
    return nc, in_maps, order


def _assemble(results, order):
    out_sorted = np.empty((NW, E), dtype=np.float32)
    for c in range(N_CORES):
        oc = results[c]["out"]
        for i in range(NBLK):
            g0 = (i * N_CORES + c) * 128
            out_sorted[g0 : g0 + 128] = oc[i * 128 : (i + 1) * 128]
    result = np.empty((NW, E), dtype=np.float32)
    result[order] = out_sorted
    return result.reshape(B, S, E)


def kernel(word_idx, ngram_ids, ngram_counts, emb_table):
    nc, in_maps, order = _prepare(word_idx, ngram_ids, ngram_counts, emb_table)
    res = run_bass_kernel_spmd(nc, in_maps, list(range(N_CORES)))
    return _assemble(res.results, order)


# revision 9
# speedup vs baseline: 1.7768x; 1.0025x over previous
"""N-gram embedding lookup (mean of hashed n-gram embeddings per word) on 8
Trainium2 NeuronCores.

Data-parallel sharding: the 16x2048 word_idx grid is flattened and split
across 8 cores; the n-gram id table and embedding table are replicated.

The SWDGE indirect-DMA primitive gathers 128 rows (one index per partition)
per instruction at a fixed ~1.4us instruction cost, so instruction count is
the bottleneck. To minimize it, words are sorted by their n-gram count and
dealt to cores in sorted stripes of 8 blocks x 128 words: every block then
only gathers K_i = max-count-in-stripe embedding rows per word instead of
the full 24, and the per-block K_i schedule is compiled into the program
(identical across cores; SPMD). Unused slots carry n-gram id 0 whose
embedding row is zeroed host-side (padding_idx=0), so short sums are exact.

Per core, per block of 128 words (32 blocks):
  1. one indirect DMA gathers the words' meta rows (24 n-gram ids + f32 bits
     of 1/count packed as one int32[32] row per word),
  2. K_i indirect DMAs gather the 128*K_i embedding rows (512B each) from
     HBM into SBUF, word-on-partition / (k,e) on the free dim,
  3. VectorE reduces over k and multiplies by 1/count,
  4. the [128,128] result is stored; the host undoes the sort permutation.
"""

import numpy as np

import concourse.bass as bass
import concourse.tile as tile
from concourse import bacc, mybir
from concourse.bass_utils import run_bass_kernel_spmd

N_CORES = 8
B, S, E = 16, 2048, 128
V, K, NG = 32000, 24, 200000
MW = 32  # meta row width (ints per word): 24 ids + 1 recip + pad
NW = B * S  # 32768 words
N_PER_CORE = NW // N_CORES  # 4096
NBLK = N_PER_CORE // 128  # 32 blocks per core
STRIPE = N_CORES * 128  # words per stripe (one block per core)

_cache = {}


def _build(k_list):
    key = tuple(k_list)
    if key in _cache:
        return _cache[key]
    nc = bacc.Bacc("TRN2", target_bir_lowering=False, debug=False, num_devices=N_CORES)
    widx = nc.dram_tensor("widx", [128, NBLK], mybir.dt.int32, kind="ExternalInput").ap()
    meta = nc.dram_tensor("meta", [V, MW], mybir.dt.int32, kind="ExternalInput").ap()
    table = nc.dram_tensor("table", [NG, E], mybir.dt.float32, kind="ExternalInput").ap()
    out = nc.dram_tensor(
        "out", [N_PER_CORE, E], mybir.dt.float32, kind="ExternalOutput"
    ).ap()

    with tile.TileContext(nc) as tc:
        with (
            tc.tile_pool(name="widxp", bufs=1) as widxp,
            tc.tile_pool(name="metap", bufs=1) as metap,
            tc.tile_pool(name="gp", bufs=8) as gp,
            tc.tile_pool(name="outp", bufs=4) as outp,
        ):
            widx_t = widxp.tile([128, NBLK], mybir.dt.int32)
            nc.sync.dma_start(widx_t[:], widx[:])

            meta_t = metap.tile([128, NBLK * MW], mybir.dt.int32)
            for j in range(NBLK):
                nc.gpsimd.indirect_dma_start(
                    out=meta_t[:, j * MW : (j + 1) * MW],
                    out_offset=None,
                    in_=meta[:],
                    in_offset=bass.IndirectOffsetOnAxis(
                        ap=widx_t[:, j : j + 1], axis=0
                    ),
                )

            for j in range(NBLK):
                kj = k_list[j]
                g = gp.tile([128, K * E], mybir.dt.float32, tag="g")
                for k in range(kj):
                    nc.gpsimd.indirect_dma_start(
                        out=g[:, k * E : (k + 1) * E],
                        out_offset=None,
                        in_=table[:],
                        in_offset=bass.IndirectOffsetOnAxis(
                            ap=meta_t[:, j * MW + k : j * MW + k + 1], axis=0
                        ),
                    )
                # contiguous halving-tree sum over the kj slots, in place in g
                m = kj
                while m > 1:
                    h = m // 2
                    nc.vector.tensor_tensor(
                        out=g[:, 0 : h * E],
                        in0=g[:, 0 : h * E],
                        in1=g[:, (m - h) * E : m * E],
                        op=mybir.AluOpType.add,
                    )
                    m -= h
                o = outp.tile([128, E], mybir.dt.float32, tag="o")
                nc.vector.tensor_scalar_mul(
                    out=o[:],
                    in0=g[:, :E],
                    scalar1=meta_t[:, j * MW + K : j * MW + K + 1].bitcast(
                        mybir.dt.float32
                    ),
                )
                nc.sync.dma_start(out[j * 128 : (j + 1) * 128, :], o[:])

    nc.compile()
    _cache[key] = nc
    return nc


def _prep_shared(ngram_ids, ngram_counts, emb_table):
    meta_np = np.zeros((V, MW), dtype=np.int32)
    meta_np[:, :K] = ngram_ids
    meta_np[:, K] = (1.0 / ngram_counts.astype(np.float32)).view(np.int32)
    table_np = np.ascontiguousarray(emb_table, dtype=np.float32).copy()
    table_np[0, :] = 0.0
    return meta_np, table_np


def _prepare(word_idx, ngram_ids, ngram_counts, emb_table):
    """Compile (count-schedule-specific) program + build per-core inputs."""
    word_idx = np.asarray(word_idx, dtype=np.int32)
    ngram_ids = np.asarray(ngram_ids, dtype=np.int32)
    ngram_counts = np.asarray(ngram_counts, dtype=np.int32)
    emb_table = np.asarray(emb_table, dtype=np.float32)

    meta_np, table_np = _prep_shared(ngram_ids, ngram_counts, emb_table)

    flat = word_idx.reshape(-1)
    cnts = ngram_counts[flat]
    order = np.argsort(-cnts, kind="stable")
    sorted_words = flat[order]
    # stripe i = global blocks [i*8, i*8+8); core c gets block i*8+c as its
    # i-th block. The compiled K for block-slot i is the stripe max = the
    # count of the stripe's first word (descending sort).
    k_list = [int(cnts[order[i * STRIPE]]) for i in range(NBLK)]

    nc = _build(k_list)

    in_maps = []
    for c in range(N_CORES):
        w = np.empty((128, NBLK), dtype=np.int32)
        for i in range(NBLK):
            g0 = (i * N_CORES + c) * 128
            w[:, i] = sorted_words[g0 : g0 + 128]
        in_maps.append({"widx": w, "meta": meta_np, "table": table_np})
    return nc, in_maps, order


def _assemble(results, order):
    out_sorted = np.empty((NW, E), dtype=np.float32)
    for c in range(N_CORES):
        oc = results[c]["out"]
        for i in range(NBLK):
            g0 = (i * N_CORES + c) * 128
            out_sorted[g0 : g0 + 128] = oc[i * 128 : (i + 1) * 128]
    result = np.empty((NW, E), dtype=np.float32)
    result[order] = out_sorted
    return result.reshape(B, S, E)


def kernel(word_idx, ngram_ids, ngram_counts, emb_table):
    nc, in_maps, order = _prepare(word_idx, ngram_ids, ngram_counts, emb_table)
    try:
        res = run_bass_kernel_spmd(nc, in_maps, list(range(N_CORES)))
    except Exception:
        # transient device hiccups recover on retry
        res = run_bass_kernel_spmd(nc, in_maps, list(range(N_CORES)))
    return _assemble(res.results, order)
